# revision 9
# baseline (speedup 1.0000x reference)
"""GATv2 2-layer GNN on 8 trn2 NeuronCores (Bass/Tile).

Sharding: destination nodes across 8 cores (NPC = NBLK*128 rows each, node
space padded to N_PAD); edges partitioned by destination owner, sorted by
(dst_block, src_half).  Per layer:
  phase A: node transforms xl = x @ Wl written to per-core DRAM tables (two
           row-halves so gather indices fit int16), xr = x_local @ Wr.
  edges:   dma_gather xl[src], xr[dst] per (block, half); m = xl + xr;
           leaky_relu (Prelu alpha=0.2); logits = reduce(m*att) per head;
           ex = exp(logits) (no max subtraction; logits are O(1));
           unnormalized softmax aggregation: num[d] = sum ex*xl, den[d] =
           sum ex via one-hot scatter matmuls accumulated in PSUM per block.
  epilogue: out = num / max(den, tiny) + bias (+ ELU after layer 1).
Between layers: PE-transpose of local h, AllGather h^T across the 8 cores.
"""
import sys
sys.path.insert(0, "/opt/trn_rl_repo")
import numpy as np

import concourse.bass as bass
import concourse.mybir as mybir
import concourse.tile as tile
import concourse.bacc as bacc
from concourse.masks import make_identity
from concourse.bass_utils import run_bass_kernel_spmd

F32 = mybir.dt.float32
I16 = mybir.dt.int16
AF = mybir.ActivationFunctionType
ALU = mybir.AluOpType

P = 128
NCORES = 8

# problem sizes (hardcoded per contract; overridable for sim tests)
N = 50000
E = 800000
C = 128          # channels (= HEADS*HID for layer 1)
H1 = 4           # heads layer 1
NEG = 0.2
NBLK = 49        # dst blocks per core
DEN_EPS = 1e-30
RUN_MODE = "hw"  # "hw" | "sim"
import os
STAGE = int(os.environ.get("KGAT_STAGE", "3"))  # 0=phaseA 1=+L1edges 2=+collective 3=full

def _derived():
    global NPC, N_PAD, NT, HALF_ROWS, C1
    NPC = NBLK * P
    N_PAD = NCORES * NPC
    NT = N_PAD // P
    HALF_ROWS = N_PAD // 2
    C1 = C // H1
_derived()


def _cdiv(a, b):
    return (a + b - 1) // b


# ----------------------------------------------------------------------------
# host-side edge preprocessing
# ----------------------------------------------------------------------------

def preprocess_edges(edge_index):
    src = edge_index[0].astype(np.int64)
    dst = edge_index[1].astype(np.int64)
    core = dst // NPC
    per_core = []
    max_cnt = 0
    for c in range(NCORES):
        m = core == c
        s, d = src[m], dst[m]
        dloc = d - c * NPC
        blk = dloc >> 7
        half = (s >= HALF_ROWS).astype(np.int64)
        key = blk * 2 + half
        order = np.argsort(key, kind="stable")
        s, dloc, key = s[order], dloc[order], key[order]
        cnt = np.bincount(key, minlength=NBLK * 2)
        max_cnt = max(max_cnt, int(cnt.max()))
        per_core.append((s, dloc, key, cnt))
    SUBH = max(1, _cdiv(max_cnt, P))
    tot = SUBH * P
    out = []
    for c in range(NCORES):
        s, dloc, key, cnt = per_core[c]
        starts = np.zeros(NBLK * 2, dtype=np.int64)
        starts[1:] = np.cumsum(cnt)[:-1]
        rank = np.arange(len(s)) - np.repeat(starts, cnt)
        pos = key * tot + rank
        nslots = NBLK * 2 * tot
        idx_xl = np.zeros(nslots, dtype=np.int16)
        idx_xr = np.zeros(nslots, dtype=np.int16)
        dst_col = np.full(nslots, -1.0, dtype=np.float32)
        half = key & 1
        idx_xl[pos] = (s - half[...] * HALF_ROWS).astype(np.int16)
        idx_xr[pos] = dloc.astype(np.int16)
        dst_col[pos] = (dloc & 127).astype(np.float32)

        def wrap(a):
            # per (block,half) region: idx j -> [j%16, j//16]; replicate x8
            a = a.reshape(NBLK * 2, tot)
            r2 = np.empty((16, NBLK * 2 * (tot // 16)), a.dtype)
            for k in range(NBLK * 2):
                r2[:, k * (tot // 16):(k + 1) * (tot // 16)] = \
                    a[k].reshape(tot // 16, 16).T
            return np.ascontiguousarray(np.tile(r2, (8, 1)))

        out.append({
            "idx_xl": wrap(idx_xl),
            "idx_xr": wrap(idx_xr),
            "dst_col": np.ascontiguousarray(
                dst_col.reshape(NBLK * 2 * SUBH, P).T),
        })
    return out, SUBH


# ----------------------------------------------------------------------------
# device program
# ----------------------------------------------------------------------------

def build_program(SUBH):
    nc = bacc.Bacc("TRN2", target_bir_lowering=False, debug=False,
                   num_devices=NCORES)

    xT = nc.dram_tensor("xT", [P, N_PAD], F32, kind="ExternalInput")
    xT_loc = nc.dram_tensor("xT_loc", [P, NPC], F32, kind="ExternalInput")
    w1l = nc.dram_tensor("w1l", [P, P], F32, kind="ExternalInput")
    w1r = nc.dram_tensor("w1r", [P, P], F32, kind="ExternalInput")
    w2l = nc.dram_tensor("w2l", [P, P], F32, kind="ExternalInput")
    w2r = nc.dram_tensor("w2r", [P, P], F32, kind="ExternalInput")
    att1_t = nc.dram_tensor("att1_t", [P, 512], F32, kind="ExternalInput")
    att2_t = nc.dram_tensor("att2_t", [P, 512], F32, kind="ExternalInput")
    b1_t = nc.dram_tensor("b1_t", [P, P], F32, kind="ExternalInput")
    b2_t = nc.dram_tensor("b2_t", [P, P], F32, kind="ExternalInput")
    iota_t = nc.dram_tensor("iota_t", [P, 512], F32, kind="ExternalInput")
    NIW = NBLK * 2 * SUBH * 8
    idx_xl_d = nc.dram_tensor("idx_xl", [P, NIW], I16, kind="ExternalInput")
    idx_xr_d = nc.dram_tensor("idx_xr", [P, NIW], I16, kind="ExternalInput")
    dst_col_d = nc.dram_tensor("dst_col", [P, NBLK * 2 * SUBH], F32,
                               kind="ExternalInput")
    out_loc = nc.dram_tensor("out_loc", [NPC, P], F32, kind="ExternalOutput")

    TOT = SUBH * P
    GRP = []
    r = SUBH
    while r > 0:
        GRP.append(min(4, r))
        r -= GRP[-1]

    with tile.TileContext(nc) as tc:
        with tc.tile_pool(name="consts", bufs=1) as cpool, \
             tc.tile_pool(name="sbuf", bufs=3) as sb, \
             tc.tile_pool(name="gath", bufs=2) as gp, \
             tc.tile_pool(name="stash", bufs=1) as stp, \
             tc.tile_pool(name="dram", bufs=1, space="DRAM") as dp, \
             tc.tile_pool(name="pa", bufs=3, space="PSUM") as pa, \
             tc.tile_pool(name="pe", bufs=3, space="PSUM") as pe, \
             tc.tile_pool(name="pt", bufs=2, space="PSUM") as pt:

            # internal DRAM (pool tiles so Tile tracks RAW deps)
            xl1_lo = dp.tile([HALF_ROWS, P], F32, tag="xl1_lo")
            xl1_hi = dp.tile([HALF_ROWS, P], F32, tag="xl1_hi")
            xr1_d = dp.tile([NPC, P], F32, tag="xr1")
            xl2_lo = dp.tile([HALF_ROWS, P], F32, tag="xl2_lo")
            xl2_hi = dp.tile([HALF_ROWS, P], F32, tag="xl2_hi")
            xr2_d = dp.tile([NPC, P], F32, tag="xr2")
            hT_loc = dp.tile([P, NPC], F32, tag="hT_loc")
            hT8 = dp.tile([NCORES * P, NPC], F32, tag="hT8")

            def cload(dram, shape, dt=F32):
                t = cpool.tile(shape, dt, tag=dram.name)
                nc.sync.dma_start(out=t[:], in_=dram[:, :])
                return t
            w1l_s = cload(w1l, [P, P]); w1r_s = cload(w1r, [P, P])
            w2l_s = cload(w2l, [P, P]); w2r_s = cload(w2r, [P, P])
            att1_s = cload(att1_t, [P, 512]); att2_s = cload(att2_t, [P, 512])
            b1_s = cload(b1_t, [P, P]); b2_s = cload(b2_t, [P, P])
            iota_s = cload(iota_t, [P, 512])
            idc = cload(dst_col_d, [P, NBLK * 2 * SUBH])
            ixl = cload(idx_xl_d, [P, NIW], I16)
            ixr = cload(idx_xr_d, [P, NIW], I16)
            ident = cpool.tile([P, P], F32, tag="ident")
            make_identity(nc, ident[:])

            # ---- phase A over a 2D [128, n_tiles*128] source AP ----
            def phase_a_flat(src_ap, n_tiles, wl_sb, dd, row_base):
                for q in range(_cdiv(n_tiles, 4)):
                    w = min(4, n_tiles - q * 4)
                    xq = sb.tile([P, 512], F32, tag="xq")
                    nc.sync.dma_start(out=xq[:, :w * P],
                                      in_=src_ap[:, q * 512:q * 512 + w * P])
                    ps = pa.tile([P, 512], F32)
                    for j in range(w):
                        nc.tensor.matmul(out=ps[:, j * P:(j + 1) * P],
                                         lhsT=xq[:, j * P:(j + 1) * P],
                                         rhs=wl_sb[:], start=True, stop=True)
                    esc = sb.tile([P, 512], F32, tag="esc")
                    nc.vector.tensor_copy(out=esc[:, :w * P], in_=ps[:, :w * P])
                    r0 = row_base + q * 512
                    nc.sync.dma_start(
                        out=dd[r0:r0 + w * P, :].rearrange(
                            "(j p) c -> p j c", p=P),
                        in_=esc[:, :w * P].rearrange("p (j c) -> p j c", c=P))

            # layer-1 xl tables from xT
            phase_a_flat(xT[:, :HALF_ROWS], NT // 2, w1l_s, xl1_lo, 0)
            phase_a_flat(xT[:, HALF_ROWS:], NT // 2, w1l_s, xl1_hi, 0)
            phase_a_flat(xT_loc[:, :], NBLK, w1r_s, xr1_d, 0)

            # ---- edge phase ----
            def edge_phase(lo_dram, hi_dram, xr_dram, att_sb, heads, stash):
                W = C + heads
                for b in range(NBLK):
                    nps = pe.tile([P, 512], F32)
                    for h2 in range(2):
                        k = b * 2 + h2
                        xlg = gp.tile([P, TOT], F32, tag="xlg")
                        nc.gpsimd.dma_gather(
                            out_ap=xlg[:].rearrange("p (n d) -> p n d", d=P),
                            in_ap=(lo_dram if h2 == 0 else hi_dram)[:, :],
                            idxs_ap=ixl[:, k * SUBH * 8:(k + 1) * SUBH * 8],
                            num_idxs=TOT, num_idxs_reg=TOT, elem_size=P,
                            single_packet=False)
                        xrg = gp.tile([P, TOT], F32, tag="xrg")
                        nc.gpsimd.dma_gather(
                            out_ap=xrg[:].rearrange("p (n d) -> p n d", d=P),
                            in_ap=xr_dram[:, :],
                            idxs_ap=ixr[:, k * SUBH * 8:(k + 1) * SUBH * 8],
                            num_idxs=TOT, num_idxs_reg=TOT, elem_size=P,
                            single_packet=False)
                        S = gp.tile([P, TOT], F32, tag="S")
                        pl = gp.tile([P, SUBH * W], F32, tag="pl")
                        ex = sb.tile([P, SUBH * heads], F32, tag="ex")
                        si = 0
                        for g in GRP:
                            FD = g * P
                            c0 = si * P
                            nc.vector.tensor_tensor(
                                out=S[:, c0:c0 + FD].rearrange(
                                    "p (s d) -> p s d", d=P),
                                in0=idc[:, k * SUBH + si:k * SUBH + si + g]
                                    .to_broadcast([P, g, P]),
                                in1=iota_s[:, :FD].rearrange(
                                    "p (s d) -> p s d", d=P),
                                op=ALU.is_equal)
                            m = sb.tile([P, 512], F32, tag="m")
                            nc.vector.tensor_tensor(
                                out=m[:, :FD], in0=xlg[:, c0:c0 + FD],
                                in1=xrg[:, c0:c0 + FD], op=ALU.add)
                            lr = sb.tile([P, 512], F32, tag="lr")
                            nc.scalar.activation(out=lr[:, :FD], in_=m[:, :FD],
                                                 func=AF.Prelu, alpha=NEG)
                            tt = sb.tile([P, 512], F32, tag="tt")
                            nc.vector.tensor_tensor(
                                out=tt[:, :FD], in0=lr[:, :FD],
                                in1=att_sb[:, :FD], op=ALU.mult)
                            lg = sb.tile([P, SUBH * heads], F32, tag="lg")
                            nc.vector.reduce_sum(
                                out=lg[:, si * heads:(si + g) * heads]
                                    .rearrange("p (s h) -> p s h", h=heads),
                                in_=tt[:, :FD].rearrange(
                                    "p (s h c) -> p s h c",
                                    h=heads, c=C // heads),
                                axis=mybir.AxisListType.X)
                            nc.scalar.activation(
                                out=ex[:, si * heads:(si + g) * heads],
                                in_=lg[:, si * heads:(si + g) * heads],
                                func=AF.Exp)
                            nc.vector.tensor_tensor(
                                out=pl[:, si * W:(si + g) * W]
                                    .rearrange("p (s w) -> p s w", w=W)
                                    [:, :, 0:C]
                                    .rearrange("p s (h c) -> p s h c", h=heads),
                                in0=xlg[:, c0:c0 + FD].rearrange(
                                    "p (s h c) -> p s h c", h=heads,
                                    c=C // heads),
                                in1=ex[:, si * heads:(si + g) * heads]
                                    .rearrange("p (s h) -> p s h", h=heads)
                                    .to_broadcast([P, g, heads, C // heads]),
                                op=ALU.mult)
                            nc.scalar.copy(
                                out=pl[:, si * W:(si + g) * W]
                                    .rearrange("p (s w) -> p s w", w=W)
                                    [:, :, C:W],
                                in_=ex[:, si * heads:(si + g) * heads]
                                    .rearrange("p (s h) -> p s h", h=heads))
                            si += g
                        for i in range(SUBH):
                            nc.tensor.matmul(
                                out=nps[:, 0:W],
                                lhsT=S[:, i * P:(i + 1) * P],
                                rhs=pl[:, i * W:(i + 1) * W],
                                start=(h2 == 0 and i == 0),
                                stop=(h2 == 1 and i == SUBH - 1))
                    nc.vector.tensor_copy(out=stash[:, b * W:(b + 1) * W],
                                          in_=nps[:, 0:W])

            # ---- layer 1 ----
            W1 = C + H1
            stash1 = stp.tile([P, NBLK * (C + H1)], F32, tag="stash")
            if STAGE >= 1:
                edge_phase(xl1_lo, xl1_hi, xr1_d, att1_s, H1, stash1)
            else:
                nc.vector.memset(stash1[:], 1.0)

            st3 = stash1[:].rearrange("p (b w) -> p b w", w=W1)
            den1 = sb.tile([P, NBLK * H1], F32, tag="den")
            nc.vector.tensor_scalar_max(out=den1[:].rearrange(
                "p (b h) -> p b h", h=H1), in0=st3[:, :, C:W1], scalar1=DEN_EPS)
            rc1 = sb.tile([P, NBLK * H1], F32, tag="rc")
            nc.vector.reciprocal(out=rc1[:], in_=den1[:])
            hsb = stp.tile([P, NPC], F32, tag="hsb")
            nc.vector.tensor_tensor(
                out=hsb[:].rearrange("p (b h c) -> p b h c", h=H1, c=C1),
                in0=st3[:, :, 0:C].rearrange("p b (h c) -> p b h c", h=H1),
                in1=rc1[:].rearrange("p (b h) -> p b h", h=H1)
                    .to_broadcast([P, NBLK, H1, C1]),
                op=ALU.mult)
            nc.vector.tensor_tensor(
                out=hsb[:].rearrange("p (b c) -> p b c", c=C),
                in0=hsb[:].rearrange("p (b c) -> p b c", c=C),
                in1=b1_s[:].rearrange("p (o c) -> p o c", o=1)
                    .to_broadcast([P, NBLK, C]),
                op=ALU.add)
            # ELU chunks
            CH = NPC // 8 if NPC % 8 == 0 else NPC
            for j in range(NPC // CH):
                sl = hsb[:, j * CH:(j + 1) * CH]
                ee = sb.tile([P, CH], F32, tag="elu_e")
                nc.scalar.activation(out=ee[:], in_=sl, func=AF.Exp)
                nc.vector.tensor_scalar_sub(out=ee[:], in0=ee[:], scalar1=1.0)
                mk = sb.tile([P, CH], mybir.dt.uint8, tag="elu_m")
                nc.vector.tensor_scalar(out=mk[:], in0=sl, scalar1=0.0,
                                        scalar2=None, op0=ALU.is_gt)
                nc.vector.copy_predicated(out=ee[:], mask=mk[:], data=sl)
                nc.vector.tensor_copy(out=sl, in_=ee[:])
            # transpose h -> hT_loc
            for b in range(NBLK):
                psT = pt.tile([P, P], F32)
                nc.tensor.transpose(out=psT[:], in_=hsb[:, b * P:(b + 1) * P],
                                    identity=ident[:])
                escT = sb.tile([P, P], F32, tag="escT")
                nc.vector.tensor_copy(out=escT[:], in_=psT[:])
                nc.sync.dma_start(out=hT_loc[:, b * P:(b + 1) * P], in_=escT[:])

            if STAGE >= 2:
                nc.gpsimd.collective_compute(
                    "AllGather", ALU.bypass,
                    replica_groups=[list(range(NCORES))],
                    ins=[hT_loc[:].opt()], outs=[hT8[:].opt()])
            else:
                for rk in range(NCORES):
                    nc.sync.dma_start(out=hT8[rk * P:(rk + 1) * P, :],
                                      in_=hT_loc[:, :])

            # ---- layer 2 node tables ----
            for rk in range(NCORES):
                half = xl2_lo if rk < NCORES // 2 else xl2_hi
                phase_a_flat(hT8[rk * P:(rk + 1) * P, :], NBLK, w2l_s,
                             half, (rk % (NCORES // 2)) * NPC)
            phase_a_flat(hT_loc[:, :], NBLK, w2r_s, xr2_d, 0)

            # ---- layer 2 ----
            W2 = C + 1
            stash2 = stp.tile([P, NBLK * (C + H1)], F32, tag="stash")
            if STAGE >= 3:
                edge_phase(xl2_lo, xl2_hi, xr2_d, att2_s, 1, stash2)
            else:
                nc.vector.memset(stash2[:], 1.0)

            st32 = stash2[:, :NBLK * W2].rearrange("p (b w) -> p b w", w=W2)
            den2 = sb.tile([P, NBLK], F32, tag="den")
            nc.vector.tensor_scalar_max(
                out=den2[:].rearrange("p (b o) -> p b o", o=1),
                in0=st32[:, :, C:W2], scalar1=DEN_EPS)
            rc2 = sb.tile([P, NBLK], F32, tag="rc")
            nc.vector.reciprocal(out=rc2[:], in_=den2[:])
            osb = stp.tile([P, NPC], F32, tag="hsb")
            nc.vector.tensor_tensor(
                out=osb[:].rearrange("p (b c) -> p b c", c=C),
                in0=st32[:, :, 0:C],
                in1=rc2[:].to_broadcast([P, NBLK, C]),
                op=ALU.mult)
            nc.vector.tensor_tensor(
                out=osb[:].rearrange("p (b c) -> p b c", c=C),
                in0=osb[:].rearrange("p (b c) -> p b c", c=C),
                in1=b2_s[:].rearrange("p (o c) -> p o c", o=1)
                    .to_broadcast([P, NBLK, C]),
                op=ALU.add)
            nc.sync.dma_start(
                out=out_loc[:, :].rearrange("(b s) c -> s b c", s=P),
                in_=osb[:].rearrange("p (b c) -> p b c", c=C))

    nc.compile()
    return nc


# ----------------------------------------------------------------------------
# entry
# ----------------------------------------------------------------------------

_cache = {}


def _build_in_maps(x, W1l, W1r, att1, b1, W2l, W2r, att2, b2, per_core):
    xT = np.zeros((P, N_PAD), np.float32)
    xT[:, :N] = np.asarray(x, np.float32).T
    att1f = np.asarray(att1, np.float32).reshape(-1)
    att2f = np.asarray(att2, np.float32).reshape(-1)
    common = {
        "xT": xT,
        "w1l": np.ascontiguousarray(np.asarray(W1l, np.float32)),
        "w1r": np.ascontiguousarray(np.asarray(W1r, np.float32)),
        "w2l": np.ascontiguousarray(np.asarray(W2l, np.float32)),
        "w2r": np.ascontiguousarray(np.asarray(W2r, np.float32)),
        "att1_t": np.ascontiguousarray(np.tile(att1f, (P, 4))),
        "att2_t": np.ascontiguousarray(np.tile(att2f, (P, 4))),
        "b1_t": np.ascontiguousarray(
            np.tile(np.asarray(b1, np.float32).reshape(1, -1), (P, 1))),
        "b2_t": np.ascontiguousarray(
            np.tile(np.asarray(b2, np.float32).reshape(1, -1), (P, 1))),
        "iota_t": np.ascontiguousarray(
            np.tile(np.arange(P, dtype=np.float32), (P, 4))),
    }
    in_maps = []
    for c in range(NCORES):
        mm = dict(common)
        mm["xT_loc"] = np.ascontiguousarray(xT[:, c * NPC:(c + 1) * NPC])
        mm.update(per_core[c])
        in_maps.append(mm)
    return in_maps


def _run_sim(nc, in_maps):
    from concourse import bass_interp
    sim = bass_interp.MultiCoreSim(nc, NCORES)
    for c in range(NCORES):
        for name, arr in in_maps[c].items():
            sim.cores[c].tensor(name)[:] = arr
    sim.simulate()
    results = []
    for c in range(NCORES):
        results.append({"out_loc": np.array(sim.cores[c].tensor("out_loc"))})
    class R:  # minimal stand-in
        pass
    r = R(); r.results = results; r.exec_time_ns = None
    return r


def kernel(x, edge_index, W1l, W1r, att1, b1, W2l, W2r, att2, b2):
    per_core, SUBH = preprocess_edges(np.asarray(edge_index))
    in_maps = _build_in_maps(x, W1l, W1r, att1, b1, W2l, W2r, att2, b2,
                             per_core)
    key = (SUBH, N, NBLK, STAGE)
    if key not in _cache:
        _cache[key] = build_program(SUBH)
    nc = _cache[key]
    if RUN_MODE == "sim":
        res = _run_sim(nc, in_maps)
    else:
        res = run_bass_kernel_spmd(nc, in_maps, list(range(NCORES)))
    kernel._last_results = res
    parts = []
    for c in range(NCORES):
        rows = min(NPC, N - c * NPC)
        parts.append(res.results[c]["out_loc"][:rows])
    return np.concatenate(parts, axis=0)
